# revision 13
# baseline (speedup 1.0000x reference)
"""SimGCN (4-layer GCN, mean-pooled [256] output) on 8 Trainium2 cores.

Sharding (per the hint): nodes/features sharded 8 ways; edges partitioned
by destination node so each core owns the scatter-add for its node shard;
the dinv-prescaled feature table is all-gathered each layer; the 64x64
weights are replicated.

Per layer, per core:
  - table x~ = dinv * x_(l-1) (own shard) -> AllGather -> full table.
  - dma_gather 256B source rows per edge (edges sorted by
    (src-quarter, dst-block), padded to 128-edge columns).
  - scatter-add via one-hot matmuls on the tensor engine: for each
    128-edge column, onehot[e, dstlane] = (dstlo[e] == lane) built by a
    vector-engine compare; PSUM accumulates per dst-block; results are
    added into an SBUF accumulator (race-free, exact).
  - out = dinv[dst] * acc + dinv^2 * x_prev (norm factorization:
    norm_e = dinv[src] dinv[dst] becomes a table pre-scale + result
    post-scale); then x_l = out @ W_l + b_l via PE
    transpose/blockdiag-matmul/transpose; per-layer feature sums
    accumulated for the means.
Final: AllReduce of per-core [4, 64] partial sums -> means -> [256].
"""
import numpy as np
from contextlib import ExitStack

import concourse.bass as bass
import concourse.tile as tile
from concourse import bacc, mybir
from concourse.masks import make_identity

N = 100000
NC = 8
SHARD = 12500
PADN = 12544
NBLK = 98
NPAD = PADN - SHARD
QROWS = PADN * NC // 4   # 25088
P = 128
D = 64
JPC = 32                 # j-columns per gather call (4096 idxs)
F32 = mybir.dt.float32
I16 = mybir.dt.int16

_CACHE = {}


def _wrap16(idx_flat):
    n = len(idx_flat)
    w = idx_flat.reshape(n // 16, 16).T.astype(np.int16)
    return np.tile(w, (8, 1))


def _make_runner(nc, n_cores):
    import jax
    from jax.sharding import Mesh, PartitionSpec
    from jax.experimental.shard_map import shard_map
    from concourse import bass2jax

    bass2jax.install_neuronx_cc_hook()
    partition_name = nc.partition_id_tensor.name if nc.partition_id_tensor else None
    in_names, out_names, out_avals, zero_outs = [], [], [], []
    for alloc in nc.m.functions[0].allocations:
        if not isinstance(alloc, mybir.MemoryLocationSet):
            continue
        name = alloc.memorylocations[0].name
        if alloc.kind == "ExternalInput":
            if name != partition_name:
                in_names.append(name)
        elif alloc.kind == "ExternalOutput":
            out_names.append(name)
            shape = tuple(alloc.tensor_shape)
            dtype = mybir.dt.np(alloc.dtype)
            out_avals.append(jax.core.ShapedArray(shape, dtype))
            zero_outs.append(np.zeros(shape, dtype))
    n_params = len(in_names)
    n_outs = len(out_avals)
    all_in = list(in_names) + list(out_names)
    if partition_name is not None:
        all_in.append(partition_name)
    donate = tuple(range(n_params, n_params + n_outs))

    def _body(*args):
        operands = list(args)
        if partition_name is not None:
            operands.append(bass2jax.partition_id_tensor())
        outs = bass2jax._bass_exec_p.bind(
            *operands, out_avals=tuple(out_avals), in_names=tuple(all_in),
            out_names=tuple(out_names), lowering_input_output_aliases=(),
            sim_require_finite=True, sim_require_nnan=True, nc=nc)
        return tuple(outs)

    devices = jax.devices()[:n_cores]
    mesh = Mesh(np.asarray(devices), ("core",))
    jitted = jax.jit(
        shard_map(_body, mesh=mesh,
                  in_specs=(PartitionSpec("core"),) * (n_params + n_outs),
                  out_specs=(PartitionSpec("core"),) * n_outs,
                  check_rep=False),
        donate_argnums=donate, keep_unused=True)
    global _LAST
    _LAST = dict(jitted=jitted, in_names=in_names, out_names=out_names,
                 out_avals=out_avals, zero_outs=zero_outs, mesh=mesh)

    def run(in_maps):
        concat_in = [np.concatenate([np.asarray(in_maps[c][n])
                                     for c in range(n_cores)], axis=0)
                     for n in in_names]
        concat_zeros = [np.zeros((n_cores * z.shape[0], *z.shape[1:]), z.dtype)
                        for z in zero_outs]
        out_arrs = jitted(*concat_in, *concat_zeros)
        jax.block_until_ready(out_arrs)
        return [{n: np.asarray(out_arrs[i]).reshape(n_cores, *out_avals[i].shape)[c]
                 for i, n in enumerate(out_names)} for c in range(n_cores)]

    return run


def _prep(edge_index):
    """Uniform-schedule edge layout.

    Returns (schedule, per-core arrays). schedule = list over j-columns of
    (quarter, block, start, stop) identical for all cores.
    """
    src = np.asarray(edge_index[0], dtype=np.int64)
    dst = np.asarray(edge_index[1], dtype=np.int64)
    deg_all = np.bincount(dst, minlength=N).astype(np.float32)

    # per-core, per-(q, b) edge lists
    groups = [[[None] * NBLK for _ in range(4)] for _ in range(NC)]
    for c in range(NC):
        lo = SHARD * c
        em = (dst >= lo) & (dst < lo + SHARD)
        es, ed = src[em], dst[em] - lo
        gpos = PADN * (es // SHARD) + (es % SHARD)
        q = gpos // QROWS
        lidx = gpos - q * QROWS
        b = ed // P
        dlo = ed % P
        key = q * NBLK + b
        order = np.argsort(key, kind="stable")
        q, lidx, b, dlo = q[order], lidx[order], b[order], dlo[order]
        bounds = np.searchsorted(key[order], np.arange(4 * NBLK + 1) * 1.0 - 0.5)
        for qq in range(4):
            for bb in range(NBLK):
                k = qq * NBLK + bb
                s, e = bounds[k], bounds[k + 1]
                groups[c][qq][bb] = (lidx[s:e], dlo[s:e])

    # uniform column counts: max over cores per (q, b)
    ncols = np.zeros((4, NBLK), np.int64)
    for qq in range(4):
        for bb in range(NBLK):
            mx = max(len(groups[c][qq][bb][0]) for c in range(NC))
            ncols[qq, bb] = max(1, -(-mx // P))

    # schedule: quarter-major; pad each quarter's total cols to JPC multiple
    schedule = []   # (q, b, start, stop) per j-column; b=-1 for filler
    for qq in range(4):
        for bb in range(NBLK):
            n = int(ncols[qq, bb])
            for j in range(n):
                schedule.append((qq, bb, j == 0, j == n - 1))
        while sum(1 for s in schedule if s[0] == qq) % JPC != 0:
            schedule.append((qq, -1, True, True))
    ntot = len(schedule)

    # per-core gather idx + dstlo arrays following the schedule
    per_core = []
    for c in range(NC):
        gi = np.zeros((ntot, P), np.int64)          # pad: row 0
        dlo_arr = np.full((ntot, P), 255.0, np.float32)
        pos = {}
        for t, (qq, bb, _, _) in enumerate(schedule):
            if bb < 0:
                continue
            j = pos.get((qq, bb), 0)
            pos[(qq, bb)] = j + 1
            li, dl = groups[c][qq][bb]
            seg = li[j * P:(j + 1) * P]
            ds = dl[j * P:(j + 1) * P]
            gi[t, :len(seg)] = seg
            dlo_arr[t, :len(ds)] = ds
        gidx = _wrap16(gi.reshape(-1))              # [P, ntot*8]
        dstlo = dlo_arr.T.copy()                    # [P, ntot]
        lo = SHARD * c
        rl = np.zeros(PADN, np.float32)
        rl[:SHARD] = deg_all[lo:lo + SHARD]
        mask = np.zeros(PADN, np.float32)
        mask[:SHARD] = 1.0
        per_core.append(dict(
            gidx=gidx, dstlo=dstlo,
            rowlen=rl.reshape(NBLK, P).T.copy(),
            mask=mask.reshape(NBLK, P).T.copy()))
    return schedule, per_core


def _build(schedule, reps=1):
    ntot = len(schedule)
    assert ntot % JPC == 0
    ncalls = ntot // JPC
    # quarter of each call (calls are quarter-pure by construction)
    call_q = [schedule[k * JPC][0] for k in range(ncalls)]

    nc = bacc.Bacc("TRN2", target_bir_lowering=False, debug=False,
                   enable_asserts=True, num_devices=NC)
    x_in = nc.dram_tensor("x_in", [P, NBLK, D], F32, kind="ExternalInput")
    gidx_in = nc.dram_tensor("gidx", [P, ntot * 8], I16, kind="ExternalInput")
    dstlo_in = nc.dram_tensor("dstlo", [P, ntot], F32, kind="ExternalInput")
    rowlen_in = nc.dram_tensor("rowlen", [P, NBLK], F32, kind="ExternalInput")
    mask_in = nc.dram_tensor("mask", [P, NBLK], F32, kind="ExternalInput")
    W_in = [nc.dram_tensor(f"W{l+1}", [D, D], F32, kind="ExternalInput")
            for l in range(4)]
    b_in = [nc.dram_tensor(f"b{l+1}", [D], F32, kind="ExternalInput")
            for l in range(4)]
    out_t = nc.dram_tensor("out", [4, D], F32, kind="ExternalOutput")

    tsh = [nc.dram_tensor(f"tsh{l}", [PADN, D], F32, kind="Internal")
           for l in range(4)]
    tfull = [nc.dram_tensor(f"tfull{l}", [PADN * NC, D], F32, kind="Internal",
                            addr_space="Shared") for l in range(4)]
    vsh = nc.dram_tensor("vsh", [4, D], F32, kind="Internal")
    vred = nc.dram_tensor("vred", [4, D], F32, kind="Internal",
                          addr_space="Shared")

    with tile.TileContext(nc) as tc, ExitStack() as ctx:
        consts = ctx.enter_context(tc.tile_pool(name="consts", bufs=1))
        sbuf = ctx.enter_context(tc.tile_pool(name="sbuf", bufs=1))
        small = ctx.enter_context(tc.tile_pool(name="small", bufs=3))
        msgp = ctx.enter_context(tc.tile_pool(name="msgp", bufs=4))
        psum = ctx.enter_context(tc.tile_pool(name="psum", bufs=1, space="PSUM"))
        psc = ctx.enter_context(tc.tile_pool(name="psc", bufs=4, space="PSUM"))

        gidx_t = consts.tile([P, ntot * 8], I16)
        nc.sync.dma_start(gidx_t[:], gidx_in.ap())
        dstlo_t = consts.tile([P, ntot], F32)
        nc.sync.dma_start(dstlo_t[:], dstlo_in.ap())

        Wt, bt = [], []
        for l in range(4):
            w = consts.tile([P, P], F32, tag=f"W{l}")
            nc.vector.memset(w[:], 0.0)
            nc.sync.dma_start(w[0:D, 0:D], W_in[l].ap())
            nc.sync.dma_start(w[D:P, D:P], W_in[l].ap())
            Wt.append(w)
            b = consts.tile([P, 1], F32, tag=f"b{l}")
            nc.sync.dma_start(b[0:D, :], b_in[l].ap()[:, None])
            nc.sync.dma_start(b[D:P, :], b_in[l].ap()[:, None])
            bt.append(b)

        ident = consts.tile([P, P], F32)
        make_identity(nc, ident[:])
        iota_i = consts.tile([P, P], mybir.dt.int32)
        nc.gpsimd.iota(iota_i[:], pattern=[[1, P]], base=0, channel_multiplier=0)
        iota_f = consts.tile([P, P], F32)
        nc.vector.tensor_copy(iota_f[:], iota_i[:])

        rl = small.tile([P, NBLK], F32, tag="tmp")
        nc.sync.dma_start(rl[:], rowlen_in.ap())
        msk = small.tile([P, NBLK], F32, tag="tmp2")
        nc.sync.dma_start(msk[:], mask_in.ap())
        deg = small.tile([P, NBLK], F32, tag="tmp3")
        nc.scalar.add(deg[:], rl[:], 1.0)
        sq = small.tile([P, NBLK], F32, tag="tmp5")
        nc.scalar.activation(sq[:], deg[:], mybir.ActivationFunctionType.Sqrt)
        dinv_r = small.tile([P, NBLK], F32, tag="tmp4")
        nc.vector.reciprocal(dinv_r[:], sq[:])
        dinv = consts.tile([P, NBLK], F32)
        nc.vector.tensor_tensor(out=dinv[:], in0=dinv_r[:], in1=msk[:],
                                op=mybir.AluOpType.mult)
        dinv2 = consts.tile([P, NBLK], F32)
        nc.vector.tensor_tensor(out=dinv2[:], in0=dinv[:], in1=dinv[:],
                                op=mybir.AluOpType.mult)

        xbufA = consts.tile([P, NBLK, D], F32, tag="xA")
        xbufB = consts.tile([P, NBLK, D], F32, tag="xB")
        xbuf = [xbufA, xbufB]

        macc = consts.tile([P, 4], F32)

        for rep in range(reps):
          nc.sync.dma_start(xbuf[0][:], x_in.ap())
          nc.vector.memset(macc[:], 0.0)
          for l in range(4):
            xprev = xbuf[l % 2]
            xnew = xbuf[(l + 1) % 2]
            # table shard x~ = dinv * xprev, per block
            xt = sbuf.tile([P, NBLK, D], F32, tag="xt")
            for bb in range(NBLK):
                nc.vector.tensor_tensor(
                    out=xt[:, bb, :], in0=xprev[:, bb, :],
                    in1=dinv[:, bb:bb + 1].to_broadcast([P, D]),
                    op=mybir.AluOpType.mult)
            nc.sync.dma_start(
                tsh[l].ap().rearrange("(j p) d -> p j d", p=P), xt[:])
            nc.gpsimd.collective_compute(
                "AllGather", mybir.AluOpType.bypass,
                replica_groups=[list(range(NC))],
                ins=[tsh[l].ap()], outs=[tfull[l].ap()])

            pacc = sbuf.tile([P, NBLK, D], F32, tag="pacc")
            nc.vector.memset(pacc[:], 0.0)

            for k in range(ncalls):
                qq = call_q[k]
                msg = msgp.tile([P, JPC, D], F32, tag="msg")
                nc.gpsimd.dma_gather(
                    out_ap=msg[:],
                    in_ap=tfull[l].ap()[qq * QROWS:(qq + 1) * QROWS, :],
                    idxs_ap=gidx_t[:, k * JPC * 8:(k + 1) * JPC * 8],
                    num_idxs=JPC * P, num_idxs_reg=JPC * P, elem_size=D,
                    single_packet=False)
                for jj in range(JPC):
                    t = k * JPC + jj
                    _, bb, st, sp = schedule[t]
                    oh = small.tile([P, P], F32, tag="oh")
                    nc.vector.tensor_scalar(
                        out=oh[:], in0=iota_f[:],
                        scalar1=dstlo_t[:, t:t + 1], scalar2=None,
                        op0=mybir.AluOpType.is_equal)
                    if st:
                        pb = psc.tile([P, D], F32, tag="pb")
                    nc.tensor.matmul(pb[:], lhsT=oh[:], rhs=msg[:, jj, :],
                                     start=st, stop=sp)
                    if sp and bb >= 0:
                        nc.vector.tensor_tensor(
                            out=pacc[:, bb, :], in0=pacc[:, bb, :], in1=pb[:],
                            op=mybir.AluOpType.add)

            # epilogue per block-pair
            for g in range(NBLK // 2):
                prop = small.tile([P, 2, D], F32, tag="prop")
                for i, bb in enumerate((2 * g, 2 * g + 1)):
                    nc.vector.tensor_tensor(
                        out=prop[:, i, :], in0=pacc[:, bb, :],
                        in1=dinv[:, bb:bb + 1].to_broadcast([P, D]),
                        op=mybir.AluOpType.mult)
                    st2 = small.tile([P, D], F32, tag="selft")
                    nc.vector.tensor_tensor(
                        out=st2[:], in0=xprev[:, bb, :],
                        in1=dinv2[:, bb:bb + 1].to_broadcast([P, D]),
                        op=mybir.AluOpType.mult)
                    nc.vector.tensor_tensor(
                        out=prop[:, i, :], in0=prop[:, i, :], in1=st2[:],
                        op=mybir.AluOpType.add)
                pT_ps = psum.tile([P, P], F32, tag="ps1")
                nc.tensor.transpose(
                    pT_ps[:], prop[:].rearrange("p t d -> p (t d)"), ident[:])
                pT = small.tile([P, P], F32, tag="pT")
                nc.vector.tensor_copy(pT[:], pT_ps[:])
                xT_ps = psum.tile([P, P], F32, tag="ps2")
                nc.tensor.matmul(xT_ps[:], lhsT=Wt[l][:], rhs=pT[:],
                                 start=True, stop=True)
                xT = small.tile([P, P], F32, tag="xT")
                nc.vector.tensor_scalar(
                    out=xT[:], in0=xT_ps[:], scalar1=bt[l][:], scalar2=None,
                    op0=mybir.AluOpType.add)
                red = small.tile([P, 1], F32, tag="red")
                nc.vector.tensor_reduce(
                    out=red[:], in_=xT[:], axis=mybir.AxisListType.X,
                    op=mybir.AluOpType.add)
                nc.vector.tensor_tensor(
                    out=macc[:, l:l + 1], in0=macc[:, l:l + 1], in1=red[:],
                    op=mybir.AluOpType.add)
                xn_ps = psum.tile([P, P], F32, tag="ps3")
                nc.tensor.transpose(xn_ps[:], xT[:], ident[:])
                nc.vector.tensor_copy(
                    xnew[:].rearrange("p j d -> p (j d)")[:, g * 2 * D:(g + 1) * 2 * D],
                    xn_ps[:])

        mT_ps = psum.tile([4, P], F32, tag="ps4")
        nc.tensor.transpose(mT_ps[:], macc[:], ident[:])
        mT_sb = small.tile([4, P], F32, tag="mTsb")
        nc.vector.tensor_copy(mT_sb[:], mT_ps[:])
        msum = small.tile([4, D], F32, tag="msum")
        nc.vector.tensor_tensor(out=msum[:], in0=mT_sb[:, 0:D],
                                in1=mT_sb[:, D:P], op=mybir.AluOpType.add)
        nc.sync.dma_start(vsh.ap(), msum[:])
        nc.gpsimd.collective_compute(
            "AllReduce", mybir.AluOpType.add,
            replica_groups=[list(range(NC))],
            ins=[vsh.ap()], outs=[vred.ap()])
        vall = small.tile([4, D], F32, tag="vall")
        nc.sync.dma_start(vall[:], vred.ap())
        bmat = small.tile([4, D], F32, tag="bmat")
        for l in range(4):
            nc.sync.dma_start(bmat[l:l + 1, :], b_in[l].ap()[None, :])
        bpad = small.tile([4, D], F32, tag="bpad")
        nc.scalar.mul(bpad[:], bmat[:], float(NPAD))
        mfin = small.tile([4, D], F32, tag="mfin")
        nc.vector.tensor_tensor(out=mfin[:], in0=vall[:], in1=bpad[:],
                                op=mybir.AluOpType.subtract)
        nc.scalar.mul(mfin[:], mfin[:], 1.0 / N)
        nc.sync.dma_start(out_t.ap(), mfin[:])

    nc.compile()
    return nc


_PREP_CACHE = {}


def _make_in_maps(inputs, per_core):
    x = np.asarray(inputs["x"], dtype=np.float32)
    in_maps = []
    for c in range(NC):
        lo = SHARD * c
        xs = np.zeros((PADN, D), np.float32)
        xs[:SHARD] = x[lo:lo + SHARD]
        x_t = xs.reshape(NBLK, P, D).transpose(1, 0, 2).copy()
        m = per_core[c]
        d = {"x_in": x_t, "gidx": m["gidx"], "dstlo": m["dstlo"],
             "rowlen": m["rowlen"], "mask": m["mask"]}
        for l in range(1, 5):
            d[f"W{l}"] = np.asarray(inputs[f"W{l}"], np.float32)
            d[f"b{l}"] = np.asarray(inputs[f"b{l}"], np.float32)
        in_maps.append(d)
    return in_maps


def kernel(x, edge_index, W1, b1, W2, b2, W3, b3, W4, b4):
    pk = id(edge_index)
    if pk not in _PREP_CACHE:
        _PREP_CACHE.clear()
        _PREP_CACHE[pk] = _prep(edge_index)
    schedule, per_core = _PREP_CACHE[pk]

    in_maps = _make_in_maps(
        {"x": x, "W1": W1, "b1": b1, "W2": W2, "b2": b2,
         "W3": W3, "b3": b3, "W4": W4, "b4": b4}, per_core)

    key = tuple(schedule)
    if key not in _CACHE:
        nc = _build(schedule)
        _CACHE[key] = _make_runner(nc, NC)
    res = _CACHE[key](in_maps)
    return res[0]["out"].reshape(256).astype(np.float32)
